# revision 10
# baseline (speedup 1.0000x reference)
"""Multi-head self-attention kernel for Trainium2 (Bass/Tile), batch-parallel
across 8 NeuronCores.

Problem: B=8, N=1024, D=768, H=12 heads, head_dim=64, fp32.
  q = x@wq+bq; k = x@wk+bk; v = x@wv+bv (per-head split)
  out = softmax(q k^T / 8) v, concat, @wo + bo

Sharding: batch dim (B=8) across the 8 cores; each core runs one full
attention for its batch element. No collectives.

Per-core dataflow (all matmuls in float32r — fp32 data rounded to the PE's
fast fp32 mode, ~1e-4 rounding):
  xT[d,n]   via PE transpose of x
  QT,KT     [768,1024] = w.T @ xT    (heads pair-packed along partitions)
  V         [1024, 12*65] natural, a ones column appended per head
  per head h, per k-chunk: S^T[k,q] = KhT.T @ QhT  (K=64 row-packed pairs)
  E^T = exp(S^T/8)  (ACT, no max-subtraction: scores are ~N(0,1) so exp is
        safe in fp32; softmax is shift-invariant so result matches reference)
  [O^T; colsum] [65,1024] += V_aug.T @ E^T  accumulated over k-chunks in PSUM
  O_norm^T = O^T * (1/sum) broadcast via K=1 matmul + fast reciprocal
  out[q,d] = O_norm^T.T @ wo  accumulated over the 6 head-pair chunks
"""

import sys

for _p in ("/opt/trn_rl_repo", "/root/.axon_site/_ro/trn_rl_repo"):
    if _p not in sys.path:
        sys.path.append(_p)

import numpy as np

import concourse.bass as bass  # noqa: F401  (import keeps bass registered)
import concourse.tile as tile
from concourse import bacc, mybir
from concourse.bass_utils import run_bass_kernel_spmd
from concourse.masks import make_identity

F32 = mybir.dt.float32
F32R = mybir.dt.float32r

B, N, D, H = 8, 1024, 768, 12
PD = D // H  # 64
DK = D // 128  # 6 contraction chunks
NQ = N // 128  # 8 sequence chunks
NPAIR = H // 2  # 6 head pairs
SCALE = 1.0 / np.sqrt(np.float32(PD))  # 0.125

_CACHE = {}
_DEBUG = False  # when True, _emit adds intermediate dumps as extra outputs


def _build(with_bias):
    nc = bacc.Bacc(None, target_bir_lowering=False)
    x_d = nc.dram_tensor("x", [N, D], F32, kind="ExternalInput")
    w_d = {
        w: nc.dram_tensor(w, [D, D], F32, kind="ExternalInput")
        for w in ("wq", "wk", "wv", "wo")
    }
    if with_bias:
        b_d = {
            b: nc.dram_tensor(b, [1, D], F32, kind="ExternalInput")
            for b in ("bq", "bk", "bv")
        }
    out_d = nc.dram_tensor("out", [N, D], F32, kind="ExternalOutput")

    with tile.TileContext(nc) as tc:
        _emit(nc, tc, x_d, w_d, b_d if with_bias else None, out_d)
    nc.compile()
    return nc


def _emit(nc, tc, x_d, w_d, b_d, out_d):
    import contextlib

    Exp = mybir.ActivationFunctionType.Exp
    est = contextlib.ExitStack()
    with est:
        # ---------- long-lived pools ----------
        const = est.enter_context(tc.tile_pool(name="const", bufs=1))
        qkp = est.enter_context(tc.tile_pool(name="qkp", bufs=1))
        vp = est.enter_context(tc.tile_pool(name="vp", bufs=1))

        ident = const.tile([128, 128], F32)
        make_identity(nc, ident)
        # ones for the denominator-broadcast matmul; used as a [1,64] slice at
        # partition 64 (the sums row's lane — engines are lane-locked)
        onesP_f = const.tile([65, 64], F32)
        nc.vector.memset(onesP_f, 1.0)
        onesP_r = const.tile([65, 64], F32R)
        nc.gpsimd.tensor_copy(onesP_r, onesP_f)
        onescol_f = const.tile([128, H], F32)
        nc.vector.memset(onescol_f, 1.0)
        if b_d is not None:
            ones512_f = const.tile([1, 512], F32)
            nc.vector.memset(ones512_f, 1.0)
            ones512_r = const.tile([1, 512], F32R)
            nc.gpsimd.tensor_copy(ones512_r, ones512_f)
            onesk_f = const.tile([1, 128], F32)
            nc.vector.memset(onesk_f, 1.0)
            onesk_r = const.tile([1, 128], F32R)
            nc.gpsimd.tensor_copy(onesk_r, onesk_f)
            bias_r = {}
            for bname in ("bq", "bk", "bv"):
                bf = const.tile([1, D], F32, tag=f"{bname}f")
                nc.sync.dma_start(out=bf, in_=b_d[bname][:, :])
                br = const.tile([1, D], F32R, tag=f"{bname}r")
                nc.gpsimd.tensor_copy(br, bf)
                bias_r[bname] = br

        def dump(name, ap, shape):
            if not _DEBUG:
                return
            d = nc.dram_tensor(name, shape, F32, kind="ExternalOutput")
            src_ap = ap if ap.dtype == F32 else ap.bitcast(F32)
            nc.sync.dma_start(out=d[:, :], in_=src_ap)

        # QT/KT: head h lives in rows (h%2)*64..+64 of tile j=h//2
        QT = [qkp.tile([128, N], F32R, tag=f"qt{j}", name=f"qt{j}") for j in range(DK)]
        KT = [qkp.tile([128, N], F32R, tag=f"kt{j}", name=f"kt{j}") for j in range(DK)]
        # V natural with ones col per head: [128, 12*65]
        V = [vp.tile([128, H * (PD + 1)], F32R, tag=f"v{i}", name=f"v{i}") for i in range(NQ)]

        # ---------- setup phase: load, round, transpose, project ----------
        with (
            tc.tile_pool(name="wr", bufs=1) as wrp,
            tc.tile_pool(name="xtp", bufs=1) as xtp,
            tc.tile_pool(name="xs", bufs=NQ) as xsp,
            tc.tile_pool(name="ws", bufs=3) as wsp,
            tc.tile_pool(name="pset", bufs=2, space="PSUM") as psset,
        ):
            # x in, fp32
            x_sb = []
            for i in range(NQ):
                t = xsp.tile([128, D], F32, tag="xs")
                nc.sync.dma_start(out=t, in_=x_d[i * 128 : (i + 1) * 128, :])
                x_sb.append(t)
            # weights in + round to f32r (gpsimd)
            w_r = {}
            for wname in ("wq", "wk", "wv"):
                w_r[wname] = []
                for j in range(DK):
                    ws = wsp.tile([128, D], F32, tag="ws")
                    nc.sync.dma_start(
                        out=ws, in_=w_d[wname][j * 128 : (j + 1) * 128, :]
                    )
                    wr = wrp.tile([128, D], F32R, tag=f"{wname}{j}")
                    nc.gpsimd.tensor_copy(wr, ws)
                    w_r[wname].append(wr)

            # transpose x -> xT (f32r), [128, 1024] x 6
            xT = [xtp.tile([128, N], F32R, tag=f"xt{j}", name=f"xt{j}") for j in range(DK)]
            for j in range(DK):
                for half in range(2):
                    pt = psset.tile([128, 512], F32, tag="tp")
                    for q in range(4):
                        i = half * 4 + q
                        nc.tensor.transpose(
                            pt[:, q * 128 : (q + 1) * 128],
                            x_sb[i][:, j * 128 : (j + 1) * 128],
                            ident,
                        )
                    nc.vector.tensor_copy(
                        xT[j][:, half * 512 : (half + 1) * 512], pt
                    )

            # QT / KT projections: QT[j] = wq[:, j-chunk].T @ xT
            for wname, dst in (("wq", QT), ("wk", KT)):
                for j in range(DK):
                    pq = psset.tile([128, N], F32, tag="proj")
                    for half in range(2):
                        hs = slice(half * 512, (half + 1) * 512)
                        for dk in range(DK):
                            nc.tensor.matmul(
                                pq[:, hs],
                                w_r[wname][dk][:, j * 128 : (j + 1) * 128],
                                xT[dk][:, hs],
                                start=(dk == 0),
                                stop=(dk == DK - 1) and b_d is None,
                            )
                        if b_d is not None:
                            bname = "bq" if wname == "wq" else "bk"
                            nc.tensor.matmul(
                                pq[:, hs],
                                bias_r[bname][:, j * 128 : (j + 1) * 128],
                                ones512_r,
                                start=False,
                                stop=True,
                            )
                    nc.vector.tensor_copy(dst[j], pq)

            # V projection (natural layout) + ones columns
            vdst3 = [
                V[i][:].rearrange("p (h c) -> p h c", c=PD + 1) for i in range(NQ)
            ]
            for i in range(NQ):
                pv = psset.tile([128, D], F32, tag="proj")
                # matmul outputs must not cross a PSUM bank (512 fp32) line
                for cs in (slice(0, 512), slice(512, 768)):
                    for dk in range(DK):
                        nc.tensor.matmul(
                            pv[:, cs],
                            xT[dk][:, i * 128 : (i + 1) * 128],
                            w_r["wv"][dk][:, cs],
                            start=(dk == 0),
                            stop=(dk == DK - 1) and b_d is None,
                        )
                    if b_d is not None:
                        nc.tensor.matmul(
                            pv[:, cs],
                            onesk_r,
                            bias_r["bv"][:, cs],
                            start=False,
                            stop=True,
                        )
                nc.vector.tensor_copy(
                    vdst3[i][:, :, 0:PD],
                    pv[:].rearrange("p (h c) -> p h c", c=PD),
                )
                nc.gpsimd.tensor_copy(
                    vdst3[i][:, :, PD : PD + 1],
                    onescol_f[:].rearrange("p (h c) -> p h c", c=1),
                )
            dump("dbg_xt0", xT[0][:], [128, N])
            dump("dbg_qt0", QT[0][:], [128, N])
            dump("dbg_kt0", KT[0][:], [128, N])
            dump("dbg_v0", V[0][:], [128, H * (PD + 1)])

        # ---------- attention + output projection ----------
        with (
            tc.tile_pool(name="wo_r", bufs=1) as wop,
            tc.tile_pool(name="op", bufs=1) as opool,
            tc.tile_pool(name="ep", bufs=3) as epool,
            tc.tile_pool(name="np_", bufs=2) as npool,
            tc.tile_pool(name="outp", bufs=3) as outp,
            tc.tile_pool(name="ws2", bufs=2) as wsp2,
            tc.tile_pool(name="psa", bufs=2, space="PSUM") as psattn,
        ):
            # wo load + round (overlaps attention)
            wo_r = []
            for j in range(DK):
                ws = wsp2.tile([128, D], F32, tag="ws2")
                nc.sync.dma_start(out=ws, in_=w_d["wo"][j * 128 : (j + 1) * 128, :])
                wr = wop.tile([128, D], F32R, tag=f"wo{j}")
                nc.gpsimd.tensor_copy(wr, ws)
                wo_r.append(wr)

            O = [opool.tile([128, N], F32R, tag=f"o{j}", name=f"opair{j}") for j in range(NPAIR)]

            for j in range(NPAIR):
                psO = [None, None]
                for kc in range(NQ):
                    psS = [None, None]
                    for sub in range(2):
                        r0 = sub * 64
                        psS[sub] = psattn.tile([128, N], F32, tag="s", name=f"psS{j}_{kc}_{sub}")
                        for half in range(2):
                            hs = slice(half * 512, (half + 1) * 512)
                            nc.tensor.matmul(
                                psS[sub][:, hs],
                                KT[j][r0 : r0 + 64, kc * 128 : (kc + 1) * 128],
                                QT[j][r0 : r0 + 64, hs],
                                start=True,
                                stop=True,
                            )
                    for sub in range(2):
                        h = 2 * j + sub
                        e = epool.tile([128, N], F32R, tag="e")
                        nc.scalar.activation(
                            out=e, in_=psS[sub], func=Exp, scale=float(SCALE)
                        )
                        if j == 0 and kc == 0 and sub == 0:
                            dump("dbg_e00", e[:], [128, N])
                        if kc == 0:
                            psO[sub] = psattn.tile([65, N], F32, tag="o", name=f"psO{j}_{sub}")
                        for half in range(2):
                            hs = slice(half * 512, (half + 1) * 512)
                            nc.tensor.matmul(
                                psO[sub][:, hs],
                                V[kc][:, h * 65 : h * 65 + 65],
                                e[:, hs],
                                start=(kc == 0),
                                stop=(kc == NQ - 1),
                            )
                if _DEBUG and j == 0:
                    dbg_oaug = npool.tile([65, N], F32, tag="dbgoaug")
                    nc.vector.tensor_copy(dbg_oaug, psO[0])
                    dump("dbg_oaug0", dbg_oaug[:], [65, N])
                # normalization per head
                for sub in range(2):
                    s65 = npool.tile([65, N], F32R, tag="sums")
                    nc.vector.tensor_copy(s65[64:65, :], psO[sub][64:65, :])
                    psR = psattn.tile([64, N], F32, tag="s")
                    for half in range(2):
                        hs = slice(half * 512, (half + 1) * 512)
                        nc.tensor.matmul(
                            psR[:, hs],
                            onesP_r[64:65, :],
                            s65[64:65, hs],
                            start=True,
                            stop=True,
                        )
                    rec = npool.tile([64, N], F32, tag="rec")
                    nc.vector.reciprocal_approx_fast(out=rec, in_=psR)
                    if _DEBUG and j == 0 and sub == 0:
                        dump("dbg_rec0", rec[:], [64, N])
                    if sub == 0:
                        nc.vector.tensor_tensor(
                            out=O[j][0:64, :],
                            in0=psO[sub][0:64, :],
                            in1=rec,
                            op=mybir.AluOpType.mult,
                        )
                    else:
                        oscr = npool.tile([64, N], F32R, tag="oscr")
                        nc.vector.tensor_tensor(
                            out=oscr,
                            in0=psO[sub][0:64, :],
                            in1=rec,
                            op=mybir.AluOpType.mult,
                        )
                        # partition shift 0..63 -> 64..127 (engines are
                        # lane-locked; DMA moves across partitions)
                        nc.sync.dma_start(out=O[j][64:128, :], in_=oscr)

            for _j in range(NPAIR):
                dump(f"dbg_opair{_j}", O[_j][:], [128, N])
            for _j in range(DK):
                dump(f"dbg_wo{_j}", wo_r[_j][:], [128, D])

            # output projection: out[q,:] = sum_j O[j][:, q-chunk].T @ wo[j]
            for qc in range(NQ):
                po = psattn.tile([128, D], F32, tag="s")
                # bank-aligned output slices (512 fp32 per PSUM bank)
                for cs in (slice(0, 512), slice(512, 768)):
                    for j in range(NPAIR):
                        nc.tensor.matmul(
                            po[:, cs],
                            O[j][:, qc * 128 : (qc + 1) * 128],
                            wo_r[j][:, cs],
                            start=(j == 0),
                            stop=(j == NPAIR - 1),
                        )
                osb = outp.tile([128, D], F32, tag="out")
                nc.vector.tensor_copy(osb, po)
                nc.sync.dma_start(
                    out=out_d[qc * 128 : (qc + 1) * 128, :], in_=osb
                )


def kernel(**inputs):
    x = np.ascontiguousarray(np.asarray(inputs["inputs"], dtype=np.float32))
    ws = {w: np.ascontiguousarray(np.asarray(inputs[w], np.float32)) for w in ("wq", "wk", "wv", "wo")}
    bs = {b: np.asarray(inputs[b], np.float32).reshape(1, D) for b in ("bq", "bk", "bv", "bo")}
    with_bias = any(np.any(bs[b]) for b in ("bq", "bk", "bv"))

    key = ("mhsa", with_bias)
    if key not in _CACHE:
        _CACHE[key] = _build(with_bias)
    nc = _CACHE[key]

    in_maps = []
    for b in range(B):
        m = {"x": np.ascontiguousarray(x[b]), **ws}
        if with_bias:
            m.update({k: bs[k] for k in ("bq", "bk", "bv")})
        in_maps.append(m)
    res = run_bass_kernel_spmd(nc, in_maps, list(range(B)))
    out = np.stack([res.results[b]["out"] for b in range(B)], axis=0)
    if np.any(bs["bo"]):
        out = out + bs["bo"][None, :, :]
    return out.astype(np.float32)


# revision 11
# speedup vs baseline: 1.2172x; 1.2172x over previous
"""Multi-head self-attention kernel for Trainium2 (Bass/Tile), batch-parallel
across 8 NeuronCores.

Problem: B=8, N=1024, D=768, H=12 heads, head_dim=64, fp32.
  q = x@wq+bq; k = x@wk+bk; v = x@wv+bv (per-head split)
  out = softmax(q k^T / 8) v, concat, @wo + bo

Sharding: batch dim (B=8) across the 8 cores; each core runs one full
attention for its batch element. No collectives.

Per-core dataflow (all matmuls in float32r — fp32 data rounded to the PE's
fast fp32 mode, ~1e-4 rounding):
  xT[d,n]   via PE transpose of x
  QT,KT     [768,1024] = w.T @ xT    (heads pair-packed along partitions)
  V         [1024, 12*65] natural, a ones column appended per head
  per head h, per k-chunk: S^T[k,q] = KhT.T @ QhT  (K=64 row-packed pairs)
  E^T = exp(S^T/8)  (ACT, no max-subtraction: scores are ~N(0,1) so exp is
        safe in fp32; softmax is shift-invariant so result matches reference)
  [O^T; colsum] [65,1024] += V_aug.T @ E^T  accumulated over k-chunks in PSUM
  O_norm^T = O^T * (1/sum) broadcast via K=1 matmul + fast reciprocal
  out[q,d] = O_norm^T.T @ wo  accumulated over the 6 head-pair chunks
"""

import sys

for _p in ("/opt/trn_rl_repo", "/root/.axon_site/_ro/trn_rl_repo"):
    if _p not in sys.path:
        sys.path.append(_p)

import numpy as np

import concourse.bass as bass  # noqa: F401  (import keeps bass registered)
import concourse.tile as tile
from concourse import bacc, mybir
from concourse.bass_utils import run_bass_kernel_spmd
from concourse.masks import make_identity

F32 = mybir.dt.float32
F32R = mybir.dt.float32r
BF16 = mybir.dt.bfloat16
import os
# matmul compute dtype for the big GEMMs: f32r (~1e-4 rounding) or bf16
# (~4e-3 rounding, but weight loads are 4x faster and overlappable)
CDT = BF16 if os.environ.get("MHSA_DTYPE", "f32r") == "bf16" else F32R

B, N, D, H = 8, 1024, 768, 12
PD = D // H  # 64
DK = D // 128  # 6 contraction chunks
NQ = N // 128  # 8 sequence chunks
NPAIR = H // 2  # 6 head pairs
SCALE = 1.0 / np.sqrt(np.float32(PD))  # 0.125

_CACHE = {}
_DEBUG = False  # when True, _emit adds intermediate dumps as extra outputs


def _build(with_bias):
    nc = bacc.Bacc(None, target_bir_lowering=False)
    x_d = nc.dram_tensor("x", [N, D], F32, kind="ExternalInput")
    w_d = {
        w: nc.dram_tensor(w, [D, D], F32, kind="ExternalInput")
        for w in ("wq", "wk", "wv", "wo")
    }
    if with_bias:
        b_d = {
            b: nc.dram_tensor(b, [1, D], F32, kind="ExternalInput")
            for b in ("bq", "bk", "bv")
        }
    out_d = nc.dram_tensor("out", [N, D], F32, kind="ExternalOutput")

    with tile.TileContext(nc) as tc:
        _emit(nc, tc, x_d, w_d, b_d if with_bias else None, out_d)
    nc.compile()
    return nc


def _emit(nc, tc, x_d, w_d, b_d, out_d):
    import contextlib

    Exp = mybir.ActivationFunctionType.Exp
    est = contextlib.ExitStack()
    with est:
        # ---------- long-lived pools ----------
        const = est.enter_context(tc.tile_pool(name="const", bufs=1))
        qkp = est.enter_context(tc.tile_pool(name="qkp", bufs=1))
        vp = est.enter_context(tc.tile_pool(name="vp", bufs=1))

        ident = const.tile([128, 128], F32)
        make_identity(nc, ident)
        # ones for the denominator-broadcast matmul; used as a [1,64] slice at
        # partition 64 (the sums row's lane — engines are lane-locked)
        onesP_f = const.tile([65, 64], F32)
        nc.vector.memset(onesP_f, 1.0)
        onesP_r = const.tile([65, 64], F32R)
        nc.gpsimd.tensor_copy(onesP_r, onesP_f)
        onescol_f = const.tile([128, H], F32)
        nc.vector.memset(onescol_f, 1.0)
        if b_d is not None:
            ones512_f = const.tile([1, 512], F32)
            nc.vector.memset(ones512_f, 1.0)
            ones512_r = const.tile([1, 512], CDT)
            nc.gpsimd.tensor_copy(ones512_r, ones512_f)
            onesk_f = const.tile([1, 128], F32)
            nc.vector.memset(onesk_f, 1.0)
            onesk_r = const.tile([1, 128], CDT)
            nc.gpsimd.tensor_copy(onesk_r, onesk_f)
            bias_r = {}
            for bname in ("bq", "bk", "bv"):
                bf = const.tile([1, D], F32, tag=f"{bname}f")
                nc.sync.dma_start(out=bf, in_=b_d[bname][:, :])
                br = const.tile([1, D], CDT, tag=f"{bname}r")
                nc.gpsimd.tensor_copy(br, bf)
                bias_r[bname] = br

        def dump(name, ap, shape):
            if not _DEBUG:
                return
            d = nc.dram_tensor(name, shape, F32, kind="ExternalOutput")
            src_ap = ap if ap.dtype == F32 else ap.bitcast(F32)
            nc.sync.dma_start(out=d[:, :], in_=src_ap)

        # QT/KT: head h lives in rows (h%2)*64..+64 of tile j=h//2
        QT = [qkp.tile([128, N], CDT, tag=f"qt{j}", name=f"qt{j}") for j in range(DK)]
        KT = [qkp.tile([128, N], CDT, tag=f"kt{j}", name=f"kt{j}") for j in range(DK)]
        # V natural with ones col per head: [128, 12*65]
        V = [vp.tile([128, H * (PD + 1)], CDT, tag=f"v{i}", name=f"v{i}") for i in range(NQ)]

        # ---------- setup phase: load, round, transpose, project ----------
        with (
            tc.tile_pool(name="wr", bufs=1) as wrp,
            tc.tile_pool(name="xtp", bufs=1) as xtp,
            tc.tile_pool(name="xs", bufs=NQ) as xsp,
            tc.tile_pool(name="ws", bufs=3) as wsp,
            tc.tile_pool(name="pset", bufs=2, space="PSUM") as psset,
        ):
            # x in, fp32
            x_sb = []
            for i in range(NQ):
                t = xsp.tile([128, D], F32, tag="xs")
                nc.sync.dma_start(out=t, in_=x_d[i * 128 : (i + 1) * 128, :])
                x_sb.append(t)
            # weights in + round to f32r (gpsimd)
            w_r = {}
            for wname in ("wq", "wk", "wv"):
                w_r[wname] = []
                for j in range(DK):
                    ws = wsp.tile([128, D], F32, tag="ws")
                    nc.sync.dma_start(
                        out=ws, in_=w_d[wname][j * 128 : (j + 1) * 128, :]
                    )
                    wr = wrp.tile([128, D], CDT, tag=f"{wname}{j}")
                    nc.gpsimd.tensor_copy(wr, ws)
                    w_r[wname].append(wr)

            # transpose x -> xT (f32r), [128, 1024] x 6
            xT = [xtp.tile([128, N], CDT, tag=f"xt{j}", name=f"xt{j}") for j in range(DK)]
            for j in range(DK):
                for half in range(2):
                    pt = psset.tile([128, 512], F32, tag="tp")
                    for q in range(4):
                        i = half * 4 + q
                        nc.tensor.transpose(
                            pt[:, q * 128 : (q + 1) * 128],
                            x_sb[i][:, j * 128 : (j + 1) * 128],
                            ident,
                        )
                    nc.vector.tensor_copy(
                        xT[j][:, half * 512 : (half + 1) * 512], pt
                    )

            # QT / KT projections: QT[j] = wq[:, j-chunk].T @ xT
            for wname, dst in (("wq", QT), ("wk", KT)):
                for j in range(DK):
                    pq = psset.tile([128, N], F32, tag="proj")
                    for half in range(2):
                        hs = slice(half * 512, (half + 1) * 512)
                        for dk in range(DK):
                            nc.tensor.matmul(
                                pq[:, hs],
                                w_r[wname][dk][:, j * 128 : (j + 1) * 128],
                                xT[dk][:, hs],
                                start=(dk == 0),
                                stop=(dk == DK - 1) and b_d is None,
                            )
                        if b_d is not None:
                            bname = "bq" if wname == "wq" else "bk"
                            nc.tensor.matmul(
                                pq[:, hs],
                                bias_r[bname][:, j * 128 : (j + 1) * 128],
                                ones512_r,
                                start=False,
                                stop=True,
                            )
                    nc.vector.tensor_copy(dst[j], pq)

            # V projection (natural layout) + ones columns
            vdst3 = [
                V[i][:].rearrange("p (h c) -> p h c", c=PD + 1) for i in range(NQ)
            ]
            for i in range(NQ):
                pv = psset.tile([128, D], F32, tag="proj")
                # matmul outputs must not cross a PSUM bank (512 fp32) line
                for cs in (slice(0, 512), slice(512, 768)):
                    for dk in range(DK):
                        nc.tensor.matmul(
                            pv[:, cs],
                            xT[dk][:, i * 128 : (i + 1) * 128],
                            w_r["wv"][dk][:, cs],
                            start=(dk == 0),
                            stop=(dk == DK - 1) and b_d is None,
                        )
                    if b_d is not None:
                        nc.tensor.matmul(
                            pv[:, cs],
                            onesk_r,
                            bias_r["bv"][:, cs],
                            start=False,
                            stop=True,
                        )
                nc.vector.tensor_copy(
                    vdst3[i][:, :, 0:PD],
                    pv[:].rearrange("p (h c) -> p h c", c=PD),
                )
                nc.gpsimd.tensor_copy(
                    vdst3[i][:, :, PD : PD + 1],
                    onescol_f[:].rearrange("p (h c) -> p h c", c=1),
                )
            dump("dbg_xt0", xT[0][:], [128, N])
            dump("dbg_qt0", QT[0][:], [128, N])
            dump("dbg_kt0", KT[0][:], [128, N])
            dump("dbg_v0", V[0][:], [128, H * (PD + 1)])

        # ---------- attention + output projection ----------
        with (
            tc.tile_pool(name="wo_r", bufs=1) as wop,
            tc.tile_pool(name="op", bufs=1) as opool,
            tc.tile_pool(name="ep", bufs=3) as epool,
            tc.tile_pool(name="np_", bufs=2) as npool,
            tc.tile_pool(name="outp", bufs=3) as outp,
            tc.tile_pool(name="ws2", bufs=2) as wsp2,
            tc.tile_pool(name="psa", bufs=2, space="PSUM") as psattn,
        ):
            # wo load + round (overlaps attention)
            wo_r = []
            for j in range(DK):
                ws = wsp2.tile([128, D], F32, tag="ws2")
                nc.sync.dma_start(out=ws, in_=w_d["wo"][j * 128 : (j + 1) * 128, :])
                wr = wop.tile([128, D], CDT, tag=f"wo{j}")
                nc.gpsimd.tensor_copy(wr, ws)
                wo_r.append(wr)

            O = [opool.tile([128, N], CDT, tag=f"o{j}", name=f"opair{j}") for j in range(NPAIR)]

            for j in range(NPAIR):
                psO = [None, None]
                for kc in range(NQ):
                    psS = [None, None]
                    for sub in range(2):
                        r0 = sub * 64
                        psS[sub] = psattn.tile([128, N], F32, tag="s", name=f"psS{j}_{kc}_{sub}")
                        for half in range(2):
                            hs = slice(half * 512, (half + 1) * 512)
                            nc.tensor.matmul(
                                psS[sub][:, hs],
                                KT[j][r0 : r0 + 64, kc * 128 : (kc + 1) * 128],
                                QT[j][r0 : r0 + 64, hs],
                                start=True,
                                stop=True,
                            )
                    for sub in range(2):
                        h = 2 * j + sub
                        e = epool.tile([128, N], CDT, tag="e")
                        nc.scalar.activation(
                            out=e, in_=psS[sub], func=Exp, scale=float(SCALE)
                        )
                        if j == 0 and kc == 0 and sub == 0:
                            dump("dbg_e00", e[:], [128, N])
                        if kc == 0:
                            psO[sub] = psattn.tile([65, N], F32, tag="o", name=f"psO{j}_{sub}")
                        for half in range(2):
                            hs = slice(half * 512, (half + 1) * 512)
                            nc.tensor.matmul(
                                psO[sub][:, hs],
                                V[kc][:, h * 65 : h * 65 + 65],
                                e[:, hs],
                                start=(kc == 0),
                                stop=(kc == NQ - 1),
                            )
                if _DEBUG and j == 0:
                    dbg_oaug = npool.tile([65, N], F32, tag="dbgoaug")
                    nc.vector.tensor_copy(dbg_oaug, psO[0])
                    dump("dbg_oaug0", dbg_oaug[:], [65, N])
                # normalization per head
                for sub in range(2):
                    s65 = npool.tile([65, N], F32R, tag="sums")
                    nc.vector.tensor_copy(s65[64:65, :], psO[sub][64:65, :])
                    psR = psattn.tile([64, N], F32, tag="s")
                    for half in range(2):
                        hs = slice(half * 512, (half + 1) * 512)
                        nc.tensor.matmul(
                            psR[:, hs],
                            onesP_r[64:65, :],
                            s65[64:65, hs],
                            start=True,
                            stop=True,
                        )
                    rec = npool.tile([64, N], F32, tag="rec")
                    nc.vector.reciprocal_approx_fast(out=rec, in_=psR)
                    if _DEBUG and j == 0 and sub == 0:
                        dump("dbg_rec0", rec[:], [64, N])
                    if sub == 0:
                        nc.vector.tensor_tensor(
                            out=O[j][0:64, :],
                            in0=psO[sub][0:64, :],
                            in1=rec,
                            op=mybir.AluOpType.mult,
                        )
                    else:
                        oscr = npool.tile([64, N], CDT, tag="oscr")
                        nc.vector.tensor_tensor(
                            out=oscr,
                            in0=psO[sub][0:64, :],
                            in1=rec,
                            op=mybir.AluOpType.mult,
                        )
                        # partition shift 0..63 -> 64..127 (engines are
                        # lane-locked; DMA moves across partitions)
                        nc.sync.dma_start(out=O[j][64:128, :], in_=oscr)

            for _j in range(NPAIR):
                dump(f"dbg_opair{_j}", O[_j][:], [128, N])
            for _j in range(DK):
                dump(f"dbg_wo{_j}", wo_r[_j][:], [128, D])

            # output projection: out[q,:] = sum_j O[j][:, q-chunk].T @ wo[j]
            for qc in range(NQ):
                po = psattn.tile([128, D], F32, tag="s")
                # bank-aligned output slices (512 fp32 per PSUM bank)
                for cs in (slice(0, 512), slice(512, 768)):
                    for j in range(NPAIR):
                        nc.tensor.matmul(
                            po[:, cs],
                            O[j][:, qc * 128 : (qc + 1) * 128],
                            wo_r[j][:, cs],
                            start=(j == 0),
                            stop=(j == NPAIR - 1),
                        )
                osb = outp.tile([128, D], F32, tag="out")
                nc.vector.tensor_copy(osb, po)
                nc.sync.dma_start(
                    out=out_d[qc * 128 : (qc + 1) * 128, :], in_=osb
                )


def kernel(**inputs):
    x = np.ascontiguousarray(np.asarray(inputs["inputs"], dtype=np.float32))
    ws = {w: np.ascontiguousarray(np.asarray(inputs[w], np.float32)) for w in ("wq", "wk", "wv", "wo")}
    bs = {b: np.asarray(inputs[b], np.float32).reshape(1, D) for b in ("bq", "bk", "bv", "bo")}
    with_bias = any(np.any(bs[b]) for b in ("bq", "bk", "bv"))

    key = ("mhsa", with_bias, str(CDT))
    if key not in _CACHE:
        _CACHE[key] = _build(with_bias)
    nc = _CACHE[key]

    in_maps = []
    for b in range(B):
        m = {"x": np.ascontiguousarray(x[b]), **ws}
        if with_bias:
            m.update({k: bs[k] for k in ("bq", "bk", "bv")})
        in_maps.append(m)
    res = run_bass_kernel_spmd(nc, in_maps, list(range(B)))
    out = np.stack([res.results[b]["out"] for b in range(B)], axis=0)
    if np.any(bs["bo"]):
        out = out + bs["bo"][None, :, :]
    return out.astype(np.float32)
